# revision 13
# baseline (speedup 1.0000x reference)
"""LIF Conv2d + STDP kernel for 8 Trainium2 NeuronCores.

Sharding: data-parallel over batch (B=8, one batch element per core).
Per timestep the two STDP weight-gradient correlations are computed
locally as (48,96)-layout matmuls and AllReduced across the 8 cores
before the (replicated) weight update.

Layout notes (per core, all fp32 unless noted):
  - x_rep3 / S3c: (48, 4096) c-major, rows (kw, c): kw-shifted copies of
    the 64x64 image with zero w-borders.  Conv over kh uses column
    offsets of +-64 (row shifts) into these flat buffers.
  - Weights live as Wk (48, 96): [(kw,c), (kh,o)] so conv lhsT slices are
    Wk[:, 32kh:32kh+32] and the dW matmuls produce the same layout.
  - LIF state (v, ref, ...) is "fold" packed (128, 1024): partition
    32*(j%4)+o, col 512*(j//4)+r for l = 512j+r, via col-tiled conv PSUM.
  - l-major tensors for dW: chunks of 128 l-positions: P3_l/S3_l
    (128, 32, 48) and Y3s/Y3p (128, 32, 96) with (kh,o) columns; kh=0/2
    groups are +-64 l-shifted copies built by SBUF->SBUF DMA.
"""

import numpy as np

T, B, C_IN, H, W_IN = 32, 8, 16, 64, 64
C_OUT, KH, KW = 32, 3, 3
L = H * W_IN  # 4096
BETA_M = float(np.exp(-1.0 / 20.0))
BETA_S = float(np.exp(-1.0 / 5.0))
BETA_PRE = float(np.exp(-1.0 / 20.0))
BETA_POST = float(np.exp(-1.0 / 20.0))
V_TH = 1.0
T_REF = 2.0
ETA = 5e-4
NORM = float(B * L)
N_CORES = 8


def _patch_tile_drain():
    """walrus in this build rejects >1 sync wait on a CTRL-class (drain)
    instruction; spread the final tile drain's waits across nops."""
    import concourse.tile as tile
    import concourse.mybir as mybir
    from concourse.vector_clock import ScopedClock

    if getattr(tile.TileContext, "_drain_patched", False):
        return

    def _drain_and_barrier(self, tick_clock, wait_clock):
        nc = self.nc
        drain_inst = nc.sync.drain()
        wait_clock.add_sem_waits(
            drain_inst.ins, ScopedClock({None: tick_clock.global_clock})
        )
        si = drain_inst.ins.sync_info
        waits = list(si.on_wait or [])
        if len(waits) > 1:
            si.on_wait = waits[:1]
            for i in range(1, len(waits)):
                nop = nc.sync.nop(nofuse=True)
                nop.ins.sync_info = mybir.SyncInfo(
                    on_wait=waits[i : i + 1], on_update=[]
                )
        nc.all_engine_barrier()
        assert self.sems is not None
        popped = nc._tile_sem_poison_stack.pop()
        assert popped is self._sem_poison
        nc.clear_and_free_semaphores(list(self.sems.allocated().values()))
        nc.all_engine_barrier()

    tile.TileContext._drain_and_barrier = _drain_and_barrier
    tile.TileContext._drain_patched = True


def _split_sync_waits(nc):
    """This walrus build accepts only ONE sync-wait slot per instruction.
    Move extra waits onto injected same-engine nops placed just before."""
    import concourse.mybir as mybir

    n = 0
    for f in nc.m.functions:
        for bb in f.blocks:
            new_insts = []
            for inst in bb.instructions:
                si = inst.sync_info
                waits = list(si.on_wait or []) if si else []
                if len(waits) > 1:
                    for w in waits[:-1]:
                        n += 1
                        nop = mybir.InstNoOp(
                            name=f"I-wsplit-{n}", engine=inst.engine,
                            ins=[], outs=[], bass_nofuse=True,
                            sync_info=mybir.SyncInfo(on_wait=[w], on_update=[]),
                        )
                        new_insts.append(nop)
                    si.on_wait = waits[-1:]
                new_insts.append(inst)
            bb.instructions = new_insts
    return n


_NC_CACHE = {}


def _build(n_steps):
    import concourse.bass as bass
    import concourse.mybir as mybir
    import concourse.tile as tile

    _patch_tile_drain()
    f32 = mybir.dt.float32
    f32r = mybir.dt.float32r
    u8 = mybir.dt.uint8
    op = mybir.AluOpType

    nc = bass.Bass("TRN2", target_bir_lowering=False, debug=False,
                   num_devices=N_CORES)

    S_d = nc.dram_tensor("S", [T, C_IN, H, W_IN], f32, kind="ExternalInput")
    W_d = nc.dram_tensor("Wk", [48, 96], f32, kind="ExternalInput")
    spk_d = nc.dram_tensor("spk_out", [T, C_OUT, L], u8, kind="ExternalOutput")
    v_d = nc.dram_tensor("v_out", [T, C_OUT, L], f32, kind="ExternalOutput")
    i_d = nc.dram_tensor("i_out", [T, C_OUT, L], f32, kind="ExternalOutput")

    cc_in = [nc.dram_tensor(f"cc_in_{t}", [128, 96], f32) for t in range(n_steps)]
    cc_out = [
        nc.dram_tensor(f"cc_out_{t}", [128, 96], f32, addr_space="Shared")
        for t in range(n_steps)
    ]

    ident48 = nc.inline_tensor(np.eye(48, dtype=np.float32), "ident48")
    ident32 = nc.inline_tensor(
        np.tile(np.eye(32, dtype=np.float32), (4, 1)), "ident32")

    with tile.TileContext(nc) as tc:
        with (
            tc.tile_pool(name="state", bufs=1) as st,
            tc.tile_pool(name="io", bufs=3) as io,
            tc.tile_pool(name="ps", bufs=2, space=bass.MemorySpace.PSUM) as ps,
            tc.tile_pool(name="psc", bufs=2, space=bass.MemorySpace.PSUM) as psc,
        ):
            # persistent state
            x3 = st.tile([48, L], f32r, tag="x3")       # x_pre kw-copies c-major
            Wk = st.tile([48, 96], f32, tag="Wk")
            v = st.tile([128, 1024], f32, tag="v")      # fold
            ref = st.tile([128, 1024], f32, tag="ref")  # fold
            P3 = st.tile([128, 32, 48], f32, tag="P3")   # pre_tr patches l-major
            Y3p = st.tile([128, 32, 96], f32, tag="Y3p")  # post_tr kh-copies l-major
            Y3s = st.tile([128, 32, 96], f32, tag="Y3s")  # spk kh-copies l-major
            i48 = st.tile([48, 48], f32, tag="i48")
            i32 = st.tile([128, 32], f32, tag="i32")

            nc.sync.dma_start(Wk[:], W_d[:])
            nc.sync.dma_start(i48[:], ident48[:])
            nc.sync.dma_start(i32[:], ident32[:])
            nc.vector.memset(v[:], 0.0)
            nc.vector.memset(ref[:], 0.0)
            nc.vector.memset(P3[:], 0.0)
            nc.vector.memset(Y3p[:], 0.0)
            nc.vector.memset(Y3s[:], 0.0)

            S_v = S_d.ap().rearrange("t c h w -> t c (h w)")

            for t in range(n_steps):
                # ---- load s(t) as 3 kw-shifted copies (zero w-borders) ----
                S3c = io.tile([48, L], f32, tag="S3c")
                s3v = S3c[:].rearrange("(k c) (h w) -> k c h w", k=3, h=H)
                # kw copy k: value at (h,w) = s[c, h, w+k-1]; border cols zero
                nc.gpsimd.memset(s3v[0, :, :, 0], 0.0)
                nc.gpsimd.memset(s3v[2, :, :, 63], 0.0)
                nc.sync.dma_start(s3v[0, :, :, 1:64], S_d[t, :, :, 0:63])
                nc.sync.dma_start(s3v[1, :, :, :], S_d[t, :, :, :])
                nc.sync.dma_start(s3v[2, :, :, 0:63], S_d[t, :, :, 1:64])

                # ---- x_pre update (must precede conv) ----
                if t == 0:
                    nc.vector.tensor_copy(x3[:], S3c[:])
                else:
                    nc.vector.scalar_tensor_tensor(
                        x3[:], x3[:], BETA_S, S3c[:], op.mult, op.add)
                Wkr = io.tile([48, 96], f32r, tag="Wkr")
                nc.scalar.copy(Wkr[:], Wk[:])

                # ---- conv: flat f32r chunks + refold copies to SBUF ----
                isyn = io.tile([128, 1024], f32, tag="isyn")
                for j in range(8):
                    flat = psc.tile([32, 512], f32, tag="fl")
                    for kh in (1, 0, 2):
                        off = 512 * j + 64 * (kh - 1)
                        lo, hi = max(off, 0), min(off + 512, L)
                        po = lo - off
                        nc.tensor.matmul(
                            flat[:, po:po + hi - lo],
                            Wkr[:, 32 * kh:32 * kh + 32],
                            x3[:, lo:hi],
                            start=(kh == 1), stop=(kh == 2),
                        )
                    g, d = j % 4, j // 4
                    nc.vector.tensor_copy(
                        isyn[32 * g:32 * g + 32, 512 * d:512 * d + 512],
                        flat[:])

                # ---- LIF chain (fold layout) ----
                v1 = io.tile([128, 1024], f32, tag="v1")
                nc.vector.scalar_tensor_tensor(
                    v1[:], v[:], BETA_M, isyn[:], op.mult, op.add)
                notref = io.tile([128, 1024], f32, tag="notref")
                nc.vector.tensor_scalar(notref[:], ref[:], 0.0, None, op.is_le)
                v2 = io.tile([128, 1024], f32, tag="v2")
                nc.vector.tensor_tensor(v2[:], v1[:], notref[:], op.mult)
                spk = io.tile([128, 1024], f32, tag="spk")
                nc.vector.tensor_scalar(spk[:], v2[:], V_TH, None, op.is_ge)
                nc.vector.scalar_tensor_tensor(
                    v[:], spk[:], 0.0, v2[:], op.is_le, op.mult)
                rrelu = io.tile([128, 1024], f32, tag="rrelu")
                nc.vector.tensor_scalar(rrelu[:], ref[:], -1.0, 0.0,
                                        op.add, op.max)
                nc.vector.scalar_tensor_tensor(
                    ref[:], spk[:], T_REF, rrelu[:], op.mult, op.add)
                spk8 = io.tile([128, 1024], u8, tag="spk8")
                nc.gpsimd.tensor_copy(spk8[:], spk[:])

                # ---- outputs (unfold: partition (g,o), col (d,r) -> [o, 512(d*4+g)+r])
                for dst_d, src in ((spk_d, spk8), (v_d, v), (i_d, isyn)):
                    dview = dst_d[t].rearrange("o (d g r) -> o d g r",
                                               g=4, r=512)
                    for g in range(4):
                        sview = src[32 * g:32 * g + 32, :].rearrange(
                            "o (d r) -> o d r", r=512)
                        nc.sync.dma_start(dview[:, :, g, :], sview)

                # ---- transposes: S3_l and SPK_l (kh=1 group of Y3s) ----
                S3l = io.tile([128, 32, 48], f32, tag="S3l")
                for k0 in range(0, 32, 4):
                    tp = ps.tile([128, 4, 48], f32, tag="tp_s")
                    tq = ps.tile([128, 4, 32], f32, tag="tp_y")
                    for k in range(k0, k0 + 4):
                        nc.tensor.matmul(
                            tp[:, k - k0, :], S3c[:, 128 * k:128 * k + 128],
                            i48[:], start=True, stop=True)
                        g, c0 = (k // 4) % 4, 512 * (k // 16) + 128 * (k % 4)
                        nc.tensor.matmul(
                            tq[:, k - k0, :],
                            spk[32 * g:32 * g + 32, c0:c0 + 128],
                            i32[32 * g:32 * g + 32, :],
                            start=True, stop=True,
                            tile_position=(32 * g, 0))
                    nc.scalar.copy(S3l[:, k0:k0 + 4, :], tp[:])
                    nc.scalar.copy(Y3s[:, k0:k0 + 4, 32:64], tq[:])

                # ---- pre-trace patch state ----
                nc.vector.scalar_tensor_tensor(
                    P3[:], P3[:], BETA_PRE, S3l[:], op.mult, op.add)

                # ---- post-trace (kh=1 cols of Y3p) ----
                nc.vector.scalar_tensor_tensor(
                    Y3p[:, :, 32:64], Y3p[:, :, 32:64], BETA_POST,
                    Y3s[:, :, 32:64], op.mult, op.add)

                # ---- kh=0 / kh=2 shifted copies via SBUF->SBUF DMA ----
                for Y in (Y3s, Y3p):
                    # kh=0: Y0[l] = Yc[l+64]
                    nc.sync.dma_start(Y[0:64, :, 0:32], Y[64:128, :, 32:64])
                    nc.sync.dma_start(Y[64:128, 0:31, 0:32], Y[0:64, 1:32, 32:64])
                    # kh=2: Y2[l] = Yc[l-64]
                    nc.sync.dma_start(Y[64:128, :, 64:96], Y[0:64, :, 32:64])
                    nc.sync.dma_start(Y[0:64, 1:32, 64:96], Y[64:128, 0:31, 32:64])

                # ---- dW matmuls: dWp = P3^T Y3s, dWm = S3l^T Y3p ----
                dps = ps.tile([128, 96], f32, tag="dps")
                for k in range(32):
                    nc.tensor.matmul(dps[0:48, :], P3[:, k, :], Y3s[:, k, :],
                                     start=(k == 0), stop=(k == 31))
                for k in range(32):
                    nc.tensor.matmul(dps[64:112, :], S3l[:, k, :], Y3p[:, k, :],
                                     start=(k == 0), stop=(k == 31),
                                     tile_position=(0, 64))

                # ---- AllReduce ----
                ccs = io.tile([128, 96], f32, tag="ccs")
                nc.scalar.copy(ccs[0:48, :], dps[0:48, :])
                nc.scalar.copy(ccs[64:112, :], dps[64:112, :])
                nc.sync.dma_start(cc_in[t][:], ccs[:])
                nc.gpsimd.collective_compute(
                    "AllReduce", op.add,
                    replica_groups=[list(range(N_CORES))],
                    ins=[cc_in[t].ap().opt()],
                    outs=[cc_out[t].ap().opt()],
                )
                ccr = io.tile([128, 96], f32, tag="ccr")
                nc.sync.dma_start(ccr[:], cc_out[t][:])

                # ---- weight update (eta/NORM folded in) ----
                eta_n = ETA / NORM
                u1 = io.tile([48, 96], f32, tag="u1")
                # u1 = eta_n*(1 - W)
                nc.vector.tensor_scalar(u1[:], Wk[:], -eta_n, eta_n,
                                        op.mult, op.add)
                t1 = io.tile([48, 96], f32, tag="t1")
                nc.vector.tensor_tensor(t1[:], u1[:], ccr[0:48, :], op.mult)
                nc.vector.tensor_tensor(Wk[:], Wk[:], t1[:], op.add)
                # W *= (1 - eta_n*dWm)
                t2 = io.tile([48, 96], f32, tag="t2")
                nc.vector.tensor_scalar(t2[:], ccr[64:112, :], -eta_n, 1.0,
                                        op.mult, op.add)
                nc.vector.tensor_tensor(Wk[:], Wk[:], t2[:], op.mult)
                nc.vector.tensor_scalar(Wk[:], Wk[:], 1.0, 0.0, op.min, op.max)

    _split_sync_waits(nc)
    return nc


def kernel(S, W0):
    from concourse import bass_utils

    S = np.ascontiguousarray(np.asarray(S, np.float32))
    W0 = np.asarray(W0, np.float32)
    Wk = np.ascontiguousarray(W0.transpose(3, 1, 2, 0).reshape(48, 96))

    key = T
    if key not in _NC_CACHE:
        _NC_CACHE[key] = _build(T)
    nc = _NC_CACHE[key]

    in_maps = [
        {"S": np.ascontiguousarray(S[:, r]), "Wk": Wk} for r in range(N_CORES)
    ]
    res = bass_utils.run_bass_kernel_spmd(nc, in_maps, core_ids=list(range(N_CORES)))
    global LAST_EXEC_NS, LAST_RES
    LAST_EXEC_NS = getattr(res, "exec_time_ns", None)
    LAST_RES = res

    spikes = np.zeros((T, B, C_OUT, H, W_IN), np.bool_)
    v_traj = np.zeros((T, B, C_OUT, H, W_IN), np.float32)
    i_traj = np.zeros((T, B, C_OUT, H, W_IN), np.float32)
    for r in range(N_CORES):
        o = res.results[r]
        spikes[:, r] = o["spk_out"].reshape(T, C_OUT, H, W_IN) != 0
        v_traj[:, r] = o["v_out"].reshape(T, C_OUT, H, W_IN)
        i_traj[:, r] = o["i_out"].reshape(T, C_OUT, H, W_IN)
    return spikes, v_traj, i_traj

